# revision 33
# baseline (speedup 1.0000x reference)
"""Multi-head attention (b=4, n=2048, dim=512, heads=8, d_head=64) on 8 TRN2 NeuronCores.

Sharding: core = 2*b + head_group. Data parallel over batch (4), tensor
parallel over heads (2 groups of 4). Each core computes the QKV projection
for its 4 heads, full attention, and a partial output projection (its
heads' rows of W_out); the host sums the two partials per batch (the
unshard step of row-parallel tensor parallelism).

Device algorithm per core (layouts chosen so no activation tensor is ever
transposed on device):
  - inputs (bf16): xT [512,2048] (pre-transposed on host), wqkT [512,512]
    (columns = q heads then k heads), wvT [512,256], woT [256,512]
  - qk^T = wqkT.T @ xT -> [512(o), 2048(n)] (partition dim = head-major d,
    exactly the q^T/k^T tiles the scores matmul needs)
  - v = xT.T @ wvT -> [2048, 256] natural layout, stored as [128, 4*65]
    tiles with a 1.0 column appended per head so P~ @ [v|1] also produces
    the softmax denominator
  - per (head, i-block of 512): S^T[j, i] = k^T.T @ q^T; P~ = exp(S^T *
    scale) on ScalarE (PSUM -> SBUF bf16, scale fused into the ACT affine)
  - PV: U^T[d+1, i] = [v|1].T @ P~ with [v|1] stationary; row d=64 is the
    denominator. Normalize: reciprocal of that row, broadcast across the 64
    d-partitions with a ones-vector matmul, multiply -> A^T [hd, n] bf16
  - out = A^T.T @ woT -> partial [2048, 512] f32 -> DMA out
"""

import functools
import sys

if "/opt/trn_rl_repo" not in sys.path:
    sys.path.insert(0, "/opt/trn_rl_repo")

import numpy as np
import ml_dtypes

import concourse.bacc as bacc
import concourse.mybir as mybir
import concourse.tile as tile
from concourse.bass_utils import run_bass_kernel_spmd

N_CORES = 8
B = 4
N = 2048          # sequence length
C = 512           # model dim
HPC = 4           # heads per core
D = 64            # head dim
SCALE = D ** -0.5

F32 = mybir.dt.float32
BF16 = mybir.dt.bfloat16

NT = N // 128     # 16 n/j tiles of 128
KT = C // 128     # 4 contraction tiles for the projections
IB = 4            # i-blocks of 512
JP = NT // 2      # 8 j-tile pairs per i-block


def _build_body(nc, tc, ctx, xT_d, wqkT_d, wvT_d, woT_d, out_d):
    sb = ctx.enter_context(tc.tile_pool(name="sb", bufs=1))
    work = ctx.enter_context(tc.tile_pool(name="work", bufs=6))
    ppool = ctx.enter_context(tc.tile_pool(name="pt", bufs=3))
    pbig = ctx.enter_context(tc.tile_pool(name="pbig", bufs=2, space="PSUM"))
    ppv = ctx.enter_context(tc.tile_pool(name="ppv", bufs=1, space="PSUM"))
    fpool = ctx.enter_context(tc.tile_pool(name="fpool", bufs=1, space="PSUM"))

    # ---- persistent SBUF tensors ----
    xT = [sb.tile([128, N], BF16, tag=f"x{k}", name=f"x{k}") for k in range(KT)]
    wqk = [sb.tile([128, 512], BF16, tag=f"wqk{k}", name=f"wqk{k}") for k in range(KT)]
    wv = [sb.tile([128, 256], BF16, tag=f"wv{k}", name=f"wv{k}") for k in range(KT)]
    wo = [sb.tile([128, 512], BF16, tag=f"wo{t}", name=f"wo{t}") for t in range(2)]
    qkT = [sb.tile([128, N], BF16, tag=f"qk{o}", name=f"qk{o}") for o in range(4)]
    vsb = [sb.tile([128, HPC * 65], BF16, tag=f"v{t}", name=f"v{t}") for t in range(NT)]
    AT = [sb.tile([128, N], BF16, tag=f"at{t}", name=f"at{t}") for t in range(2)]

    # ---- input DMAs, ordered by first use and chunked so the first qk
    # matmuls only wait on the first 512-column slice of each xT tile ----
    for k in range(KT):
        weng = nc.sync if k % 2 == 0 else nc.scalar
        xeng = nc.scalar if k % 2 == 0 else nc.sync
        weng.dma_start(out=wqk[k][:], in_=wqkT_d[k * 128:(k + 1) * 128, :])
        xeng.dma_start(
            out=xT[k][:, 0:512], in_=xT_d[k * 128:(k + 1) * 128, 0:512])
    for nch in range(1, 4):
        for k in range(KT):
            nc.sync.dma_start(
                out=xT[k][:, nch * 512:(nch + 1) * 512],
                in_=xT_d[k * 128:(k + 1) * 128, nch * 512:(nch + 1) * 512],
            )
    for k in range(KT):
        nc.sync.dma_start(out=wv[k][:], in_=wvT_d[k * 128:(k + 1) * 128, :])
    for t in range(2):
        nc.sync.dma_start(out=wo[t][:], in_=woT_d[t * 128:(t + 1) * 128, :])

    # ones columns of v tiles (never overwritten by the v eviction)
    for t in range(NT):
        v3 = vsb[t][:].rearrange("p (h c) -> p h c", c=65)
        nc.vector.memset(v3[:, :, 64:65], 1.0)

    # ---- projection emitters (interleaved into the attention pipeline) ----
    # qk o-tiles: 0 = q heads 0/1, 1 = q heads 2/3, 2 = k heads 0/1, 3 = k 2/3.
    def emit_v(ts):
        for t in ts:
            ps = fpool.tile([128, 512], F32, tag="fp", name="ps")
            for k in range(KT):
                nc.tensor.matmul(
                    ps[:, 0:256],
                    xT[k][:, t * 128:(t + 1) * 128],
                    wv[k][:],
                    start=(k == 0),
                    stop=(k == KT - 1),
                )
            v3 = vsb[t][:].rearrange("p (h c) -> p h c", c=65)
            p3 = ps[:, 0:256].rearrange("p (h c) -> p h c", c=64)
            nc.vector.tensor_copy(v3[:, :, 0:64], p3)

    # ---- attention, software-pipelined one unit deep ----
    # Emit scores+exp for unit u, then PV+normalize for unit u-1, so the
    # TensorEngine's in-order stream never parks ScalarE behind a PV chain.
    exp_t = mybir.ActivationFunctionType.Exp
    mult_t = mybir.AluOpType.mult

    def emit_s_exp(ib, h, fillers=None, interleave_pv=False):
        rows = slice((h % 2) * 64, (h % 2) * 64 + 64)
        q_t = qkT[h // 2]
        k_t = qkT[2 + h // 2]
        rhs_q = q_t[rows, ib * 512:(ib + 1) * 512]
        ptiles = []
        pu = ppv.tile([128, 512], F32, tag="pu", name="pu") if interleave_pv else None
        groups = [(g * 3, min(3, NT - g * 3)) for g in range(6)]  # 3,3,3,3,3,1
        for g, (jt0, njt) in enumerate(groups):
            width = njt * 512
            ps = pbig.tile([128, 1536], F32, tag="ps")
            for half in range(njt):
                jt = jt0 + half
                nc.tensor.matmul(
                    ps[:, half * 512:(half + 1) * 512],
                    k_t[rows, jt * 128:(jt + 1) * 128],
                    rhs_q,
                    start=True,
                    stop=True,
                )
            pt = ppool.tile([128, 1536], BF16, tag=f"pt{g}", name=f"pt{g}")
            nc.scalar.activation(pt[:, 0:width], ps[:, 0:width], exp_t, scale=SCALE)
            ptiles.append(pt)
            if fillers:
                fillers.pop(0)()
                if len(fillers) > 10:
                    fillers.pop(0)()
            if interleave_pv:
                for half in range(njt):
                    jt = jt0 + half
                    nc.tensor.matmul(
                        pu[0:65, :],
                        vsb[jt][:, h * 65:h * 65 + 65],
                        pt[:, half * 512:half * 512 + 512],
                        start=(jt == 0),
                        stop=(jt == NT - 1),
                    )
        if interleave_pv:
            finish_norm(ib, h, pu)
            return None
        return ptiles

    def finish_norm(ib, h, pu):
        rows = slice((h % 2) * 64, (h % 2) * 64 + 64)
        for c in range(2):
            cs = slice(c * 256, (c + 1) * 256)
            rc = work.tile([1, 256], F32, tag=f"rc{c}", name="rc")
            nc.vector.reciprocal(rc[:], pu[64:65, cs])
            rbc = work.tile([64, 256], F32, tag=f"rbc{c}", name="rbc")
            nc.gpsimd.partition_broadcast(rbc[:], rc[:], channels=64)
            nc.vector.tensor_tensor(
                out=AT[h // 2][rows, ib * 512 + c * 256:ib * 512 + (c + 1) * 256],
                in0=pu[0:64, cs],
                in1=rbc[:],
                op=mult_t,
            )

    def emit_pv_norm(ib, h, ptiles):
        pu = ppv.tile([128, 512], F32, tag="pu")
        for jt in range(NT):
            nc.tensor.matmul(
                pu[0:65, :],
                vsb[jt][:, h * 65:h * 65 + 65],
                ptiles[jt // 3][:, (jt % 3) * 512:(jt % 3) * 512 + 512],
                start=(jt == 0),
                stop=(jt == NT - 1),
            )
        finish_norm(ib, h, pu)

    def proj_chunk(nt):
        def f():
            pp = fpool.tile([128, 512], F32, tag="fp", name="pp")
            for t2 in range(2):
                nc.tensor.matmul(
                    pp[:, 0:512],
                    AT[t2][:, nt * 128:(nt + 1) * 128],
                    wo[t2][:],
                    start=(t2 == 0),
                    stop=(t2 == 1),
                )
            ot_s = work.tile([128, 512], F32, tag="o", name="ot_s")
            nc.vector.tensor_copy(ot_s[:], pp[:, 0:512])
            nc.sync.dma_start(out=out_d[nt * 128:(nt + 1) * 128, :], in_=ot_s[:])
        return f

    def emit_proj(ib):
        for t in range(4):
            proj_chunk(ib * 4 + t)()

    # Pipeline, h-outer: heads 0/1 only need qk tiles 0 and 2, so emit those
    # up front, then drip the v projection (needed by the first PV, unit 1)
    # and the qk 1/3 tiles (first needed by unit 8 = head 2) one small chunk
    # after each exp, so the TensorEngine fills ScalarE-paced gaps instead of
    # clumping projections between units.
    def qk_chunk(ot, nch):
        def f():
            ps = fpool.tile([128, 512], F32, tag="fp", name="ps")
            for k in range(KT):
                nc.tensor.matmul(
                    ps[:, 0:512],
                    wqk[k][:, ot * 128:(ot + 1) * 128],
                    xT[k][:, nch * 512:(nch + 1) * 512],
                    start=(k == 0),
                    stop=(k == KT - 1),
                )
            nc.vector.tensor_copy(qkT[ot][:, nch * 512:(nch + 1) * 512], ps[:, 0:512])
        return f

    qk_chunk(2, 0)()
    qk_chunk(0, 0)()
    vc = [(lambda t: (lambda: emit_v([2 * t, 2 * t + 1])))(t) for t in range(NT // 2)]
    # unit 0: rest of k heads 0/1 (all j needed by unit 1 = (ib1, h0)),
    # q chunk for ib1, early v; unit 1: remaining q, rest of v; units 2-3:
    # qk tiles 1/3 (first needed by unit 8 = head 2).
    fillers = [qk_chunk(2, 1), qk_chunk(2, 2), qk_chunk(2, 3), qk_chunk(0, 1)]
    fillers += vc[0:4]
    fillers += [qk_chunk(0, 2), qk_chunk(0, 3)] + vc[4:8]
    fillers += [qk_chunk(1, 0), qk_chunk(1, 1)]
    fillers += [qk_chunk(1, 2), qk_chunk(1, 3), qk_chunk(3, 0), qk_chunk(3, 1)]
    fillers += [qk_chunk(3, 2), qk_chunk(3, 3)]

    prev = None
    units = [(ib, h) for h in range(HPC) for ib in range(IB)]
    done_ib = {}
    last_ui = len(units) - 1
    for ui, (ib, h) in enumerate(units):
        ptiles = emit_s_exp(ib, h, fillers if fillers else None,
                            interleave_pv=(ui == last_ui))
        if prev is not None:
            emit_pv_norm(*prev)
            pib, ph = prev[0], prev[1]
            done_ib[pib] = done_ib.get(pib, 0) + 1
            if done_ib[pib] == HPC:
                fillers.extend(proj_chunk(pib * 4 + t) for t in range(4))
        prev = (ib, h, ptiles)
    if prev[2] is not None:
        emit_pv_norm(*prev)
    for f in fillers:
        f()
    emit_proj(prev[0])


@functools.lru_cache(maxsize=1)
def _build():
    nc = bacc.Bacc("TRN2", target_bir_lowering=False, debug=False,
                   num_devices=N_CORES)
    xT_d = nc.dram_tensor("xT", [C, N], BF16, kind="ExternalInput").ap()
    wqkT_d = nc.dram_tensor("wqkT", [C, 512], BF16, kind="ExternalInput").ap()
    wvT_d = nc.dram_tensor("wvT", [C, 256], BF16, kind="ExternalInput").ap()
    woT_d = nc.dram_tensor("woT", [256, C], BF16, kind="ExternalInput").ap()
    out_d = nc.dram_tensor("out", [N, C], F32, kind="ExternalOutput").ap()
    from contextlib import ExitStack
    with tile.TileContext(nc) as tc, ExitStack() as ctx:
        _build_body(nc, tc, ctx, xT_d, wqkT_d, wvT_d, woT_d, out_d)
    nc.compile()
    return nc


def _shard_inputs(x, W_qkv, W_out):
    bf16 = ml_dtypes.bfloat16
    in_maps = []
    for core in range(N_CORES):
        b, hg = core // 2, core % 2
        xT = np.ascontiguousarray(x[b].T).astype(bf16)
        rows_q = W_qkv[hg * 256:(hg + 1) * 256, :]
        rows_k = W_qkv[512 + hg * 256:512 + (hg + 1) * 256, :]
        wqkT = np.ascontiguousarray(
            np.concatenate([rows_q, rows_k], 0).T).astype(bf16)
        wvT = np.ascontiguousarray(
            W_qkv[1024 + hg * 256:1024 + (hg + 1) * 256, :].T).astype(bf16)
        woT = np.ascontiguousarray(
            W_out[:, hg * 256:(hg + 1) * 256].T).astype(bf16)
        in_maps.append({"xT": xT, "wqkT": wqkT, "wvT": wvT, "woT": woT})
    return in_maps


def _run(inputs, trace=False, tmpdir=None):
    x = np.asarray(inputs["x"], dtype=np.float32)
    W_qkv = np.asarray(inputs["W_qkv"], dtype=np.float32)
    W_out = np.asarray(inputs["W_out"], dtype=np.float32)
    nc = _build()
    in_maps = _shard_inputs(x, W_qkv, W_out)
    kwargs = {}
    if trace:
        kwargs = dict(trace=True, tmpdir=tmpdir)
    res = run_bass_kernel_spmd(nc, in_maps, core_ids=list(range(N_CORES)), **kwargs)
    out = np.zeros((B, N, C), np.float32)
    for core in range(N_CORES):
        out[core // 2] += res.results[core]["out"]
    return out, res


def kernel(**inputs):
    out, _ = _run(inputs)
    return out


# revision 34
# speedup vs baseline: 1.4980x; 1.4980x over previous
"""Multi-head attention (b=4, n=2048, dim=512, heads=8, d_head=64) on 8 TRN2 NeuronCores.

Sharding: core = 2*b + head_group. Data parallel over batch (4), tensor
parallel over heads (2 groups of 4). Each core computes the QKV projection
for its 4 heads, full attention, and a partial output projection (its
heads' rows of W_out); the host sums the two partials per batch (the
unshard step of row-parallel tensor parallelism).

Device algorithm per core (layouts chosen so no activation tensor is ever
transposed on device):
  - inputs (bf16): xT [512,2048] (pre-transposed on host), wqkT [512,512]
    (columns = q heads then k heads), wvT [512,256], woT [256,512]
  - qk^T = wqkT.T @ xT -> [512(o), 2048(n)] (partition dim = head-major d,
    exactly the q^T/k^T tiles the scores matmul needs)
  - v = xT.T @ wvT -> [2048, 256] natural layout, stored as [128, 4*65]
    tiles with a 1.0 column appended per head so P~ @ [v|1] also produces
    the softmax denominator
  - per (head, i-block of 512): S^T[j, i] = k^T.T @ q^T; P~ = exp(S^T *
    scale) on ScalarE (PSUM -> SBUF bf16, scale fused into the ACT affine)
  - PV: U^T[d+1, i] = [v|1].T @ P~ with [v|1] stationary; row d=64 is the
    denominator. Normalize: reciprocal of that row, broadcast across the 64
    d-partitions with a ones-vector matmul, multiply -> A^T [hd, n] bf16
  - out = A^T.T @ woT -> partial [2048, 512] f32 -> DMA out
"""

import functools
import sys

if "/opt/trn_rl_repo" not in sys.path:
    sys.path.insert(0, "/opt/trn_rl_repo")

import numpy as np
import ml_dtypes

import concourse.bacc as bacc
import concourse.mybir as mybir
import concourse.tile as tile
from concourse.bass_utils import run_bass_kernel_spmd

N_CORES = 8
B = 4
N = 2048          # sequence length
C = 512           # model dim
HPC = 4           # heads per core
D = 64            # head dim
SCALE = D ** -0.5

F32 = mybir.dt.float32
BF16 = mybir.dt.bfloat16

NT = N // 128     # 16 n/j tiles of 128
KT = C // 128     # 4 contraction tiles for the projections
IB = 4            # i-blocks of 512
JP = NT // 2      # 8 j-tile pairs per i-block


def _build_body(nc, tc, ctx, xT_d, wqkT_d, wvT_d, woT_d, out_d):
    sb = ctx.enter_context(tc.tile_pool(name="sb", bufs=1))
    work = ctx.enter_context(tc.tile_pool(name="work", bufs=6))
    ppool = ctx.enter_context(tc.tile_pool(name="pt", bufs=3))
    pbig = ctx.enter_context(tc.tile_pool(name="pbig", bufs=2, space="PSUM"))
    ppv = ctx.enter_context(tc.tile_pool(name="ppv", bufs=2, space="PSUM"))

    # ---- persistent SBUF tensors ----
    xT = [sb.tile([128, N], BF16, tag=f"x{k}", name=f"x{k}") for k in range(KT)]
    wqk = [sb.tile([128, 512], BF16, tag=f"wqk{k}", name=f"wqk{k}") for k in range(KT)]
    wv = [sb.tile([128, 256], BF16, tag=f"wv{k}", name=f"wv{k}") for k in range(KT)]
    wo = [sb.tile([128, 512], BF16, tag=f"wo{t}", name=f"wo{t}") for t in range(2)]
    qkT = [sb.tile([128, N], BF16, tag=f"qk{o}", name=f"qk{o}") for o in range(4)]
    vsb = [sb.tile([128, HPC * 65], BF16, tag=f"v{t}", name=f"v{t}") for t in range(NT)]
    AT = [sb.tile([128, N], BF16, tag=f"at{t}", name=f"at{t}") for t in range(2)]

    # ---- input DMAs, ordered by first use and chunked so the first qk
    # matmuls only wait on the first 512-column slice of each xT tile ----
    for k in range(KT):
        weng = nc.sync if k % 2 == 0 else nc.scalar
        xeng = nc.scalar if k % 2 == 0 else nc.sync
        weng.dma_start(out=wqk[k][:], in_=wqkT_d[k * 128:(k + 1) * 128, :])
        xeng.dma_start(
            out=xT[k][:, 0:512], in_=xT_d[k * 128:(k + 1) * 128, 0:512])
    for nch in range(1, 4):
        for k in range(KT):
            nc.sync.dma_start(
                out=xT[k][:, nch * 512:(nch + 1) * 512],
                in_=xT_d[k * 128:(k + 1) * 128, nch * 512:(nch + 1) * 512],
            )
    for k in range(KT):
        nc.sync.dma_start(out=wv[k][:], in_=wvT_d[k * 128:(k + 1) * 128, :])
    for t in range(2):
        nc.sync.dma_start(out=wo[t][:], in_=woT_d[t * 128:(t + 1) * 128, :])

    # ones columns of v tiles (never overwritten by the v eviction)
    for t in range(NT):
        v3 = vsb[t][:].rearrange("p (h c) -> p h c", c=65)
        nc.vector.memset(v3[:, :, 64:65], 1.0)

    # ---- projection emitters (interleaved into the attention pipeline) ----
    # qk o-tiles: 0 = q heads 0/1, 1 = q heads 2/3, 2 = k heads 0/1, 3 = k 2/3.
    def emit_v(ts):
        for t in ts:
            ps = pbig.tile([128, 1536], F32, tag="ps", name="ps")
            for k in range(KT):
                nc.tensor.matmul(
                    ps[:, 0:256],
                    xT[k][:, t * 128:(t + 1) * 128],
                    wv[k][:],
                    start=(k == 0),
                    stop=(k == KT - 1),
                )
            v3 = vsb[t][:].rearrange("p (h c) -> p h c", c=65)
            p3 = ps[:, 0:256].rearrange("p (h c) -> p h c", c=64)
            nc.vector.tensor_copy(v3[:, :, 0:64], p3)

    # ---- attention, software-pipelined one unit deep ----
    # Emit scores+exp for unit u, then PV+normalize for unit u-1, so the
    # TensorEngine's in-order stream never parks ScalarE behind a PV chain.
    exp_t = mybir.ActivationFunctionType.Exp
    mult_t = mybir.AluOpType.mult

    def emit_s_exp(ib, h, fillers=None, interleave_pv=False):
        rows = slice((h % 2) * 64, (h % 2) * 64 + 64)
        q_t = qkT[h // 2]
        k_t = qkT[2 + h // 2]
        rhs_q = q_t[rows, ib * 512:(ib + 1) * 512]
        ptiles = []
        pu = ppv.tile([128, 512], F32, tag="pu", name="pu") if interleave_pv else None
        groups = [(g * 3, min(3, NT - g * 3)) for g in range(6)]  # 3,3,3,3,3,1
        for g, (jt0, njt) in enumerate(groups):
            width = njt * 512
            ps = pbig.tile([128, 1536], F32, tag="ps")
            for half in range(njt):
                jt = jt0 + half
                nc.tensor.matmul(
                    ps[:, half * 512:(half + 1) * 512],
                    k_t[rows, jt * 128:(jt + 1) * 128],
                    rhs_q,
                    start=True,
                    stop=True,
                )
            pt = ppool.tile([128, 1536], BF16, tag=f"pt{g}", name=f"pt{g}")
            nc.scalar.activation(pt[:, 0:width], ps[:, 0:width], exp_t, scale=SCALE)
            ptiles.append(pt)
            if fillers:
                fillers.pop(0)()
                if len(fillers) > 10:
                    fillers.pop(0)()
            if interleave_pv:
                for half in range(njt):
                    jt = jt0 + half
                    nc.tensor.matmul(
                        pu[0:65, :],
                        vsb[jt][:, h * 65:h * 65 + 65],
                        pt[:, half * 512:half * 512 + 512],
                        start=(jt == 0),
                        stop=(jt == NT - 1),
                    )
        if interleave_pv:
            finish_norm(ib, h, pu)
            return None
        return ptiles

    def finish_norm(ib, h, pu):
        rows = slice((h % 2) * 64, (h % 2) * 64 + 64)
        for c in range(2):
            cs = slice(c * 256, (c + 1) * 256)
            rc = work.tile([1, 256], F32, tag=f"rc{c}", name="rc")
            nc.vector.reciprocal(rc[:], pu[64:65, cs])
            rbc = work.tile([64, 256], F32, tag=f"rbc{c}", name="rbc")
            nc.gpsimd.partition_broadcast(rbc[:], rc[:], channels=64)
            nc.vector.tensor_tensor(
                out=AT[h // 2][rows, ib * 512 + c * 256:ib * 512 + (c + 1) * 256],
                in0=pu[0:64, cs],
                in1=rbc[:],
                op=mult_t,
            )

    def emit_pv_norm(ib, h, ptiles):
        pu = ppv.tile([128, 512], F32, tag="pu")
        for jt in range(NT):
            nc.tensor.matmul(
                pu[0:65, :],
                vsb[jt][:, h * 65:h * 65 + 65],
                ptiles[jt // 3][:, (jt % 3) * 512:(jt % 3) * 512 + 512],
                start=(jt == 0),
                stop=(jt == NT - 1),
            )
        finish_norm(ib, h, pu)

    def proj_chunk(nt):
        def f():
            pp = pbig.tile([128, 1536], F32, tag="ps", name="pp")
            for t2 in range(2):
                nc.tensor.matmul(
                    pp[:, 0:512],
                    AT[t2][:, nt * 128:(nt + 1) * 128],
                    wo[t2][:],
                    start=(t2 == 0),
                    stop=(t2 == 1),
                )
            ot_s = work.tile([128, 512], F32, tag="o", name="ot_s")
            nc.vector.tensor_copy(ot_s[:], pp[:, 0:512])
            nc.sync.dma_start(out=out_d[nt * 128:(nt + 1) * 128, :], in_=ot_s[:])
        return f

    def emit_proj(ib):
        for t in range(4):
            proj_chunk(ib * 4 + t)()

    # Pipeline, h-outer: heads 0/1 only need qk tiles 0 and 2, so emit those
    # up front, then drip the v projection (needed by the first PV, unit 1)
    # and the qk 1/3 tiles (first needed by unit 8 = head 2) one small chunk
    # after each exp, so the TensorEngine fills ScalarE-paced gaps instead of
    # clumping projections between units.
    def qk_chunk(ot, nch):
        def f():
            ps = pbig.tile([128, 1536], F32, tag="ps", name="ps")
            for k in range(KT):
                nc.tensor.matmul(
                    ps[:, 0:512],
                    wqk[k][:, ot * 128:(ot + 1) * 128],
                    xT[k][:, nch * 512:(nch + 1) * 512],
                    start=(k == 0),
                    stop=(k == KT - 1),
                )
            nc.vector.tensor_copy(qkT[ot][:, nch * 512:(nch + 1) * 512], ps[:, 0:512])
        return f

    qk_chunk(2, 0)()
    qk_chunk(0, 0)()
    vc = [(lambda t: (lambda: emit_v([2 * t, 2 * t + 1])))(t) for t in range(NT // 2)]
    # unit 0: rest of k heads 0/1 (all j needed by unit 1 = (ib1, h0)),
    # q chunk for ib1, early v; unit 1: remaining q, rest of v; units 2-3:
    # qk tiles 1/3 (first needed by unit 8 = head 2).
    fillers = [qk_chunk(2, 1), qk_chunk(2, 2), qk_chunk(2, 3), qk_chunk(0, 1)]
    fillers += vc[0:4]
    fillers += [qk_chunk(0, 2), qk_chunk(0, 3)] + vc[4:8]
    fillers += [qk_chunk(1, 0), qk_chunk(1, 1)]
    fillers += [qk_chunk(1, 2), qk_chunk(1, 3), qk_chunk(3, 0), qk_chunk(3, 1)]
    fillers += [qk_chunk(3, 2), qk_chunk(3, 3)]

    prev = None
    units = [(ib, h) for h in range(HPC) for ib in range(IB)]
    done_ib = {}
    last_ui = len(units) - 1
    for ui, (ib, h) in enumerate(units):
        ptiles = emit_s_exp(ib, h, fillers if fillers else None,
                            interleave_pv=(ui == last_ui))
        if prev is not None:
            emit_pv_norm(*prev)
            pib, ph = prev[0], prev[1]
            done_ib[pib] = done_ib.get(pib, 0) + 1
            if done_ib[pib] == HPC:
                fillers.extend(proj_chunk(pib * 4 + t) for t in range(4))
        prev = (ib, h, ptiles)
    if prev[2] is not None:
        emit_pv_norm(*prev)
    for f in fillers:
        f()
    emit_proj(prev[0])


@functools.lru_cache(maxsize=1)
def _build():
    nc = bacc.Bacc("TRN2", target_bir_lowering=False, debug=False,
                   num_devices=N_CORES)
    xT_d = nc.dram_tensor("xT", [C, N], BF16, kind="ExternalInput").ap()
    wqkT_d = nc.dram_tensor("wqkT", [C, 512], BF16, kind="ExternalInput").ap()
    wvT_d = nc.dram_tensor("wvT", [C, 256], BF16, kind="ExternalInput").ap()
    woT_d = nc.dram_tensor("woT", [256, C], BF16, kind="ExternalInput").ap()
    out_d = nc.dram_tensor("out", [N, C], F32, kind="ExternalOutput").ap()
    from contextlib import ExitStack
    with tile.TileContext(nc) as tc, ExitStack() as ctx:
        _build_body(nc, tc, ctx, xT_d, wqkT_d, wvT_d, woT_d, out_d)
    nc.compile()
    return nc


def _shard_inputs(x, W_qkv, W_out):
    bf16 = ml_dtypes.bfloat16
    in_maps = []
    for core in range(N_CORES):
        b, hg = core // 2, core % 2
        xT = np.ascontiguousarray(x[b].T).astype(bf16)
        rows_q = W_qkv[hg * 256:(hg + 1) * 256, :]
        rows_k = W_qkv[512 + hg * 256:512 + (hg + 1) * 256, :]
        wqkT = np.ascontiguousarray(
            np.concatenate([rows_q, rows_k], 0).T).astype(bf16)
        wvT = np.ascontiguousarray(
            W_qkv[1024 + hg * 256:1024 + (hg + 1) * 256, :].T).astype(bf16)
        woT = np.ascontiguousarray(
            W_out[:, hg * 256:(hg + 1) * 256].T).astype(bf16)
        in_maps.append({"xT": xT, "wqkT": wqkT, "wvT": wvT, "woT": woT})
    return in_maps


def _run(inputs, trace=False, tmpdir=None):
    x = np.asarray(inputs["x"], dtype=np.float32)
    W_qkv = np.asarray(inputs["W_qkv"], dtype=np.float32)
    W_out = np.asarray(inputs["W_out"], dtype=np.float32)
    nc = _build()
    in_maps = _shard_inputs(x, W_qkv, W_out)
    kwargs = {}
    if trace:
        kwargs = dict(trace=True, tmpdir=tmpdir)
    res = run_bass_kernel_spmd(nc, in_maps, core_ids=list(range(N_CORES)), **kwargs)
    out = np.zeros((B, N, C), np.float32)
    for core in range(N_CORES):
        out[core // 2] += res.results[core]["out"]
    return out, res


def kernel(**inputs):
    out, _ = _run(inputs)
    return out


# revision 35
# speedup vs baseline: 1.6170x; 1.0795x over previous
"""Multi-head attention (b=4, n=2048, dim=512, heads=8, d_head=64) on 8 TRN2 NeuronCores.

Sharding: core = 2*b + head_group. Data parallel over batch (4), tensor
parallel over heads (2 groups of 4). Each core computes the QKV projection
for its 4 heads, full attention, and a partial output projection (its
heads' rows of W_out); the host sums the two partials per batch (the
unshard step of row-parallel tensor parallelism).

Device algorithm per core (layouts chosen so no activation tensor is ever
transposed on device):
  - inputs (bf16): xT [512,2048] (pre-transposed on host), wqkT [512,512]
    (columns = q heads then k heads), wvT [512,256], woT [256,512]
  - qk^T = wqkT.T @ xT -> [512(o), 2048(n)] (partition dim = head-major d,
    exactly the q^T/k^T tiles the scores matmul needs)
  - v = xT.T @ wvT -> [2048, 256] natural layout, stored as [128, 4*65]
    tiles with a 1.0 column appended per head so P~ @ [v|1] also produces
    the softmax denominator
  - per (head, i-block of 512): S^T[j, i] = k^T.T @ q^T; P~ = exp(S^T *
    scale) on ScalarE (PSUM -> SBUF bf16, scale fused into the ACT affine)
  - PV: U^T[d+1, i] = [v|1].T @ P~ with [v|1] stationary; row d=64 is the
    denominator. Normalize: reciprocal of that row, broadcast across the 64
    d-partitions with a ones-vector matmul, multiply -> A^T [hd, n] bf16
  - out = A^T.T @ woT -> partial [2048, 512] f32 -> DMA out
"""

import functools
import sys

if "/opt/trn_rl_repo" not in sys.path:
    sys.path.insert(0, "/opt/trn_rl_repo")

import numpy as np
import ml_dtypes

import concourse.bacc as bacc
import concourse.mybir as mybir
import concourse.tile as tile
from concourse.bass_utils import run_bass_kernel_spmd

N_CORES = 8
B = 4
N = 2048          # sequence length
C = 512           # model dim
HPC = 4           # heads per core
D = 64            # head dim
SCALE = D ** -0.5

F32 = mybir.dt.float32
BF16 = mybir.dt.bfloat16

NT = N // 128     # 16 n/j tiles of 128
KT = C // 128     # 4 contraction tiles for the projections
IB = 4            # i-blocks of 512
JP = NT // 2      # 8 j-tile pairs per i-block


def _build_body(nc, tc, ctx, xT_d, wqkT_d, wvT_d, woT_d, out_d):
    sb = ctx.enter_context(tc.tile_pool(name="sb", bufs=1))
    work = ctx.enter_context(tc.tile_pool(name="work", bufs=6))
    ppool = ctx.enter_context(tc.tile_pool(name="pt", bufs=3))
    pbig = ctx.enter_context(tc.tile_pool(name="pbig", bufs=3, space="PSUM"))
    ppv = ctx.enter_context(tc.tile_pool(name="ppv", bufs=2, space="PSUM"))

    # ---- persistent SBUF tensors ----
    xT = [sb.tile([128, N], BF16, tag=f"x{k}", name=f"x{k}") for k in range(KT)]
    wqk = [sb.tile([128, 512], BF16, tag=f"wqk{k}", name=f"wqk{k}") for k in range(KT)]
    wv = [sb.tile([128, 256], BF16, tag=f"wv{k}", name=f"wv{k}") for k in range(KT)]
    wo = [sb.tile([128, 512], BF16, tag=f"wo{t}", name=f"wo{t}") for t in range(2)]
    qkT = [sb.tile([128, N], BF16, tag=f"qk{o}", name=f"qk{o}") for o in range(4)]
    vsb = [sb.tile([128, HPC * 65], BF16, tag=f"v{t}", name=f"v{t}") for t in range(NT)]
    AT = [sb.tile([128, N], BF16, tag=f"at{t}", name=f"at{t}") for t in range(2)]

    # ---- input DMAs, ordered by first use and chunked so the first qk
    # matmuls only wait on the first 512-column slice of each xT tile ----
    for k in range(KT):
        weng = nc.sync if k % 2 == 0 else nc.scalar
        xeng = nc.scalar if k % 2 == 0 else nc.sync
        weng.dma_start(out=wqk[k][:], in_=wqkT_d[k * 128:(k + 1) * 128, :])
        xeng.dma_start(
            out=xT[k][:, 0:512], in_=xT_d[k * 128:(k + 1) * 128, 0:512])
    for nch in range(1, 4):
        for k in range(KT):
            nc.sync.dma_start(
                out=xT[k][:, nch * 512:(nch + 1) * 512],
                in_=xT_d[k * 128:(k + 1) * 128, nch * 512:(nch + 1) * 512],
            )
    for k in range(KT):
        nc.sync.dma_start(out=wv[k][:], in_=wvT_d[k * 128:(k + 1) * 128, :])
    for t in range(2):
        nc.sync.dma_start(out=wo[t][:], in_=woT_d[t * 128:(t + 1) * 128, :])

    # ones columns of v tiles (never overwritten by the v eviction)
    for t in range(NT):
        v3 = vsb[t][:].rearrange("p (h c) -> p h c", c=65)
        nc.vector.memset(v3[:, :, 64:65], 1.0)

    # ---- projection emitters (interleaved into the attention pipeline) ----
    # qk o-tiles: 0 = q heads 0/1, 1 = q heads 2/3, 2 = k heads 0/1, 3 = k 2/3.
    def emit_v(ts):
        for t in ts:
            ps = pbig.tile([128, 1024], F32, tag="ps", name="ps")
            for k in range(KT):
                nc.tensor.matmul(
                    ps[:, 0:256],
                    xT[k][:, t * 128:(t + 1) * 128],
                    wv[k][:],
                    start=(k == 0),
                    stop=(k == KT - 1),
                )
            v3 = vsb[t][:].rearrange("p (h c) -> p h c", c=65)
            p3 = ps[:, 0:256].rearrange("p (h c) -> p h c", c=64)
            nc.vector.tensor_copy(v3[:, :, 0:64], p3)

    # ---- attention, software-pipelined one unit deep ----
    # Emit scores+exp for unit u, then PV+normalize for unit u-1, so the
    # TensorEngine's in-order stream never parks ScalarE behind a PV chain.
    exp_t = mybir.ActivationFunctionType.Exp
    mult_t = mybir.AluOpType.mult

    def emit_s_exp(ib, h, fillers=None, prev=None, self_pv=False):
        rows = slice((h % 2) * 64, (h % 2) * 64 + 64)
        q_t = qkT[h // 2]
        k_t = qkT[2 + h // 2]
        rhs_q = q_t[rows, ib * 512:(ib + 1) * 512]
        ptiles = []
        pu = ppv.tile([128, 512], F32, tag="pu", name="pu") if self_pv else None
        if prev is not None:
            pib, ph, pptiles = prev
            ppu = ppv.tile([128, 512], F32, tag="pu", name="ppu")
        for jp in range(JP):
            ps = pbig.tile([128, 1024], F32, tag="ps")
            for half in range(2):
                jt = jp * 2 + half
                nc.tensor.matmul(
                    ps[:, half * 512:(half + 1) * 512],
                    k_t[rows, jt * 128:(jt + 1) * 128],
                    rhs_q,
                    start=True,
                    stop=True,
                )
            pt = ppool.tile([128, 1024], BF16, tag=f"pt{jp}", name=f"pt{jp}")
            nc.scalar.activation(pt[:], ps[:], exp_t, scale=SCALE)
            ptiles.append(pt)
            if fillers:
                fillers.pop(0)()
                if len(fillers) > 10:
                    fillers.pop(0)()
            if prev is not None:
                for half in range(2):
                    jt = jp * 2 + half
                    nc.tensor.matmul(
                        ppu[0:65, :],
                        vsb[jt][:, ph * 65:ph * 65 + 65],
                        pptiles[jt // 2][:, (jt % 2) * 512:(jt % 2) * 512 + 512],
                        start=(jt == 0),
                        stop=(jt == NT - 1),
                    )
            if self_pv:
                for half in range(2):
                    jt = jp * 2 + half
                    nc.tensor.matmul(
                        pu[0:65, :],
                        vsb[jt][:, h * 65:h * 65 + 65],
                        pt[:, half * 512:half * 512 + 512],
                        start=(jt == 0),
                        stop=(jt == NT - 1),
                    )
        if prev is not None:
            finish_norm(prev[0], prev[1], ppu)
        if self_pv:
            finish_norm(ib, h, pu)
            return None
        return ptiles

    def finish_norm(ib, h, pu):
        rows = slice((h % 2) * 64, (h % 2) * 64 + 64)
        for c in range(2):
            cs = slice(c * 256, (c + 1) * 256)
            rc = work.tile([1, 256], F32, tag=f"rc{c}", name="rc")
            nc.vector.reciprocal(rc[:], pu[64:65, cs])
            rbc = work.tile([64, 256], F32, tag=f"rbc{c}", name="rbc")
            nc.gpsimd.partition_broadcast(rbc[:], rc[:], channels=64)
            nc.vector.tensor_tensor(
                out=AT[h // 2][rows, ib * 512 + c * 256:ib * 512 + (c + 1) * 256],
                in0=pu[0:64, cs],
                in1=rbc[:],
                op=mult_t,
            )

    def emit_pv_norm(ib, h, ptiles):
        pu = ppv.tile([128, 512], F32, tag="pu")
        for jt in range(NT):
            nc.tensor.matmul(
                pu[0:65, :],
                vsb[jt][:, h * 65:h * 65 + 65],
                ptiles[jt // 2][:, (jt % 2) * 512:(jt % 2) * 512 + 512],
                start=(jt == 0),
                stop=(jt == NT - 1),
            )
        finish_norm(ib, h, pu)

    def proj_chunk(nt):
        def f():
            pp = pbig.tile([128, 1024], F32, tag="ps", name="pp")
            for t2 in range(2):
                nc.tensor.matmul(
                    pp[:, 0:512],
                    AT[t2][:, nt * 128:(nt + 1) * 128],
                    wo[t2][:],
                    start=(t2 == 0),
                    stop=(t2 == 1),
                )
            ot_s = work.tile([128, 512], F32, tag="o", name="ot_s")
            nc.vector.tensor_copy(ot_s[:], pp[:, 0:512])
            nc.sync.dma_start(out=out_d[nt * 128:(nt + 1) * 128, :], in_=ot_s[:])
        return f

    def emit_proj(ib):
        for t in range(4):
            proj_chunk(ib * 4 + t)()

    # Pipeline, h-outer: heads 0/1 only need qk tiles 0 and 2, so emit those
    # up front, then drip the v projection (needed by the first PV, unit 1)
    # and the qk 1/3 tiles (first needed by unit 8 = head 2) one small chunk
    # after each exp, so the TensorEngine fills ScalarE-paced gaps instead of
    # clumping projections between units.
    def qk_chunk(ot, nch):
        def f():
            ps = pbig.tile([128, 1024], F32, tag="ps", name="ps")
            for k in range(KT):
                nc.tensor.matmul(
                    ps[:, 0:512],
                    wqk[k][:, ot * 128:(ot + 1) * 128],
                    xT[k][:, nch * 512:(nch + 1) * 512],
                    start=(k == 0),
                    stop=(k == KT - 1),
                )
            nc.vector.tensor_copy(qkT[ot][:, nch * 512:(nch + 1) * 512], ps[:, 0:512])
        return f

    qk_chunk(2, 0)()
    qk_chunk(0, 0)()
    vc = [(lambda t: (lambda: emit_v([2 * t, 2 * t + 1])))(t) for t in range(NT // 2)]
    # unit 0: rest of k heads 0/1 (all j needed by unit 1 = (ib1, h0)),
    # q chunk for ib1, early v; unit 1: remaining q, rest of v; units 2-3:
    # qk tiles 1/3 (first needed by unit 8 = head 2).
    fillers = [qk_chunk(2, 1), qk_chunk(2, 2), qk_chunk(2, 3), qk_chunk(0, 1)]
    fillers += vc[0:4]
    fillers += [qk_chunk(0, 2), qk_chunk(0, 3)] + vc[4:8]
    fillers += [qk_chunk(1, 0), qk_chunk(1, 1)]
    fillers += [qk_chunk(1, 2), qk_chunk(1, 3), qk_chunk(3, 0), qk_chunk(3, 1)]
    fillers += [qk_chunk(3, 2), qk_chunk(3, 3)]

    prev = None
    units = [(ib, h) for h in range(HPC) for ib in range(IB)]
    done_ib = {}
    last_ui = len(units) - 1
    for ui, (ib, h) in enumerate(units):
        ptiles = emit_s_exp(ib, h, fillers if fillers else None,
                            prev=prev, self_pv=(ui == last_ui))
        if prev is not None:
            pib, ph = prev[0], prev[1]
            done_ib[pib] = done_ib.get(pib, 0) + 1
            if done_ib[pib] == HPC:
                fillers.extend(proj_chunk(pib * 4 + t) for t in range(4))
        prev = (ib, h, ptiles)
    for f in fillers:
        f()
    emit_proj(prev[0])


@functools.lru_cache(maxsize=1)
def _build():
    nc = bacc.Bacc("TRN2", target_bir_lowering=False, debug=False,
                   num_devices=N_CORES)
    xT_d = nc.dram_tensor("xT", [C, N], BF16, kind="ExternalInput").ap()
    wqkT_d = nc.dram_tensor("wqkT", [C, 512], BF16, kind="ExternalInput").ap()
    wvT_d = nc.dram_tensor("wvT", [C, 256], BF16, kind="ExternalInput").ap()
    woT_d = nc.dram_tensor("woT", [256, C], BF16, kind="ExternalInput").ap()
    out_d = nc.dram_tensor("out", [N, C], F32, kind="ExternalOutput").ap()
    from contextlib import ExitStack
    with tile.TileContext(nc) as tc, ExitStack() as ctx:
        _build_body(nc, tc, ctx, xT_d, wqkT_d, wvT_d, woT_d, out_d)
    nc.compile()
    return nc


def _shard_inputs(x, W_qkv, W_out):
    bf16 = ml_dtypes.bfloat16
    in_maps = []
    for core in range(N_CORES):
        b, hg = core // 2, core % 2
        xT = np.ascontiguousarray(x[b].T).astype(bf16)
        rows_q = W_qkv[hg * 256:(hg + 1) * 256, :]
        rows_k = W_qkv[512 + hg * 256:512 + (hg + 1) * 256, :]
        wqkT = np.ascontiguousarray(
            np.concatenate([rows_q, rows_k], 0).T).astype(bf16)
        wvT = np.ascontiguousarray(
            W_qkv[1024 + hg * 256:1024 + (hg + 1) * 256, :].T).astype(bf16)
        woT = np.ascontiguousarray(
            W_out[:, hg * 256:(hg + 1) * 256].T).astype(bf16)
        in_maps.append({"xT": xT, "wqkT": wqkT, "wvT": wvT, "woT": woT})
    return in_maps


def _run(inputs, trace=False, tmpdir=None):
    x = np.asarray(inputs["x"], dtype=np.float32)
    W_qkv = np.asarray(inputs["W_qkv"], dtype=np.float32)
    W_out = np.asarray(inputs["W_out"], dtype=np.float32)
    nc = _build()
    in_maps = _shard_inputs(x, W_qkv, W_out)
    kwargs = {}
    if trace:
        kwargs = dict(trace=True, tmpdir=tmpdir)
    res = run_bass_kernel_spmd(nc, in_maps, core_ids=list(range(N_CORES)), **kwargs)
    out = np.zeros((B, N, C), np.float32)
    for core in range(N_CORES):
        out[core // 2] += res.results[core]["out"]
    return out, res


def kernel(**inputs):
    out, _ = _run(inputs)
    return out
